# revision 1
# baseline (speedup 1.0000x reference)
"""EquivariantCrossAttention kernel for 8 Trainium2 NeuronCores.

Sharding strategy (per spec hint): the num_coords (N) axis of x / x_h /
output is split 8 ways across the NeuronCores; latents p/a, window_sigma
and all weights are replicated on every core, so the attention reduction
over L stays core-local and needs no collectives.

Execution: the sharded computation is jax.pmap'ed across the 8 axon
NeuronCores (PJRT -> neuronx-cc NEFF). If the device path is unavailable
in the calling environment, a bit-accurate numpy fallback runs on host.
"""

import os

os.environ.setdefault("NEURON_CC_FLAGS", "--auto-cast=none")

import numpy as np

B, N, L = 2, 2048, 128
H, D = 4, 32
A = 128
C = 2
TWO_PI = 6.283185307179586
NC = 8
NS = N // NC  # 256 coords per core

_REP_KEYS = (
    "p", "a", "window_sigma",
    "wr_q", "w1_q", "b1_q", "w2_q", "b2_q",
    "wr_v", "w1_v", "b1_v", "w2_v", "b2_v",
    "wq", "bq", "wk", "bk", "wv", "bv",
    "cf_w1", "cf_b1", "cf_g", "cf_bt", "cf_w2", "cf_b2",
    "vf_w1", "vf_b1", "vf_g", "vf_bt", "vf_w2", "vf_b2",
    "mf_w1", "mf_b1", "mf_g", "mf_bt", "mf_w2", "mf_b2",
    "wo", "bo",
)


# ---------------------------------------------------------------- jax path
_PMAP_FN = None


def _build_pmap():
    import jax
    import jax.numpy as jnp

    def _ln(h, g, b):
        mu = h.mean(-1, keepdims=True)
        var = ((h - mu) ** 2).mean(-1, keepdims=True)
        return (h - mu) * jax.lax.rsqrt(var + 1e-6) * g + b

    def _ffn(x, w1, b1, g, bt, w2, b2):
        h = jax.nn.gelu(x @ w1 + b1)
        return _ln(h, g, bt) @ w2 + b2

    def _emb(inv, wr, w1, b1, w2, b2):
        proj = TWO_PI * (inv @ wr)
        feat = jnp.concatenate([jnp.sin(proj), jnp.cos(proj)], axis=-1)
        return jax.nn.gelu(feat @ w1 + b1) @ w2 + b2

    def shard_fn(x, x_h, r):
        # x: (B, NS, C)  x_h: (B, NS, D); everything in r replicated.
        ns = x.shape[1]
        inv = x[:, :, None, :] - r["p"][:, None, :, :]           # (B,NS,L,C)
        emb_q = _emb(inv, r["wr_q"], r["w1_q"], r["b1_q"],
                     r["w2_q"], r["b2_q"])                        # (B,NS,L,D)
        k = (r["a"] @ r["wk"] + r["bk"]).reshape(B, L, H, D)
        # fold k into wq: att = emb_q @ (wq . k) -- avoids materializing
        # the (B,NS,L,H*D) query tensor (exact reassociation).
        wq3 = r["wq"].reshape(D, H, D)
        wk_f = jnp.einsum("ehd,blhd->belh", wq3, k)               # (B,D,L,H)
        bk_f = jnp.einsum("hd,blhd->blh", r["bq"].reshape(H, D), k)
        v = r["a"] @ r["wv"] + r["bv"]                            # (B,L,H*D)
        inv_emb_v = _emb(inv, r["wr_v"], r["w1_v"], r["b1_v"],
                         r["w2_v"], r["b2_v"])                    # (B,NS,L,D)
        gb = _ffn(x_h, r["cf_w1"], r["cf_b1"], r["cf_g"], r["cf_bt"],
                  r["cf_w2"], r["cf_b2"])                         # (B,NS,2D)
        g_, b_ = jnp.split(gb, 2, axis=-1)
        inv_emb_v = inv_emb_v * (1.0 + g_[:, :, None, :]) + b_[:, :, None, :]
        # vf FFN inlined so the vb half of vf_w2 folds through mf_w1 --
        # vb and the 256-wide vgb are never materialized (exact algebra).
        hv = jax.nn.gelu(inv_emb_v @ r["vf_w1"] + r["vf_b1"])
        hv = _ln(hv, r["vf_g"], r["vf_bt"])                       # (B,NS,L,D)
        vg = hv @ r["vf_w2"][:, :H * D] + r["vf_b2"][:H * D]      # (B,NS,L,HD)
        vfilm = (v[:, None, :, :] * (1.0 + vg)).reshape(B, ns, L, H, D)
        w2b = r["vf_w2"][:, H * D:].reshape(D, H, D)
        w2b_f = jnp.einsum("chd,df->chf", w2b, r["mf_w1"])
        const_f = (jnp.einsum("hd,df->hf",
                              r["vf_b2"][H * D:].reshape(H, D), r["mf_w1"])
                   + r["mf_b1"])                                  # (H,D)
        pre = (jnp.einsum("bnlhd,df->bnlhf", vfilm, r["mf_w1"])
               + jnp.einsum("bnlc,chf->bnlhf", hv, w2b_f) + const_f)
        v = _ln(jax.nn.gelu(pre), r["mf_g"], r["mf_bt"]) @ r["mf_w2"] + r["mf_b2"]
        scale = 1.0 / (D ** 0.5)
        att = (jnp.einsum("bnle,belh->bnlh", emb_q, wk_f)
               + bk_f[:, None]) * scale
        dist2 = jnp.sum(inv * inv, axis=-1)
        gw = -dist2 / (2.0 * r["window_sigma"][:, None, :, 0] ** 2)
        att = att + gw[..., None]
        att = jax.nn.softmax(att, axis=2)
        y = jnp.einsum("bnlh,bnlhd->bnhd", att, v).reshape(B, ns, H * D)
        return y @ r["wo"] + r["bo"]                              # (B,NS,D)

    devs = [d for d in jax.devices() if d.platform != "cpu"][:NC]
    if len(devs) < NC:
        raise RuntimeError(f"need {NC} accelerator devices, got {len(devs)}")
    return jax.pmap(shard_fn, in_axes=(0, 0, None), devices=devs)


def _run_jax(inputs):
    global _PMAP_FN
    if _PMAP_FN is None:
        _PMAP_FN = _build_pmap()
    x = np.ascontiguousarray(
        inputs["x"].reshape(B, NC, NS, C).transpose(1, 0, 2, 3))
    x_h = np.ascontiguousarray(
        inputs["x_h"].reshape(B, NC, NS, D).transpose(1, 0, 2, 3))
    rep = {k: inputs[k] for k in _REP_KEYS}
    y = _PMAP_FN(x, x_h, rep)                                     # (NC,B,NS,D)
    y = np.asarray(y)
    return np.ascontiguousarray(
        y.transpose(1, 0, 2, 3).reshape(B, N, D)).astype(np.float32)


# -------------------------------------------------------------- numpy path
def _gelu(x):
    # matches jax.nn.gelu(approximate=True)
    x3 = x * x * x
    return (0.5 * x * (1.0 + np.tanh(0.7978845608028654
                                     * (x + 0.044715 * x3)))).astype(np.float32)


def _ln_np(h, g, b):
    mu = h.mean(-1, keepdims=True, dtype=np.float32)
    var = ((h - mu) ** 2).mean(-1, keepdims=True, dtype=np.float32)
    return (h - mu) / np.sqrt(var + 1e-6) * g + b


def _ffn_np(x, w1, b1, g, bt, w2, b2):
    h = _gelu(x @ w1 + b1)
    return _ln_np(h, g, bt) @ w2 + b2


def _emb_np(inv, wr, w1, b1, w2, b2):
    proj = TWO_PI * (inv @ wr)
    feat = np.concatenate([np.sin(proj), np.cos(proj)], axis=-1)
    return _gelu(feat @ w1 + b1) @ w2 + b2


def _run_numpy(inputs):
    i = {k: np.asarray(v, dtype=np.float32) for k, v in inputs.items()}
    out = np.empty((B, N, D), dtype=np.float32)
    k = (i["a"] @ i["wk"] + i["bk"]).reshape(B, L, H, D)
    v0 = i["a"] @ i["wv"] + i["bv"]
    gb_full = _ffn_np(i["x_h"], i["cf_w1"], i["cf_b1"], i["cf_g"],
                      i["cf_bt"], i["cf_w2"], i["cf_b2"])
    scale = 1.0 / (D ** 0.5)
    for s in range(NC):  # per-shard to bound memory
        sl = slice(s * NS, (s + 1) * NS)
        inv = i["x"][:, sl, None, :] - i["p"][:, None, :, :]
        q = _emb_np(inv, i["wr_q"], i["w1_q"], i["b1_q"], i["w2_q"], i["b2_q"])
        q = (q @ i["wq"] + i["bq"]).reshape(B, NS, L, H, D)
        iev = _emb_np(inv, i["wr_v"], i["w1_v"], i["b1_v"], i["w2_v"], i["b2_v"])
        g_ = gb_full[:, sl, :D]
        b_ = gb_full[:, sl, D:]
        iev = iev * (1.0 + g_[:, :, None, :]) + b_[:, :, None, :]
        vgb = _ffn_np(iev, i["vf_w1"], i["vf_b1"], i["vf_g"], i["vf_bt"],
                      i["vf_w2"], i["vf_b2"])
        vg, vb = vgb[..., :H * D], vgb[..., H * D:]
        v = v0[:, None, :, :] * (1.0 + vg) + vb
        v = _ffn_np(v.reshape(B, NS, L, H, D), i["mf_w1"], i["mf_b1"],
                    i["mf_g"], i["mf_bt"], i["mf_w2"], i["mf_b2"])
        att = np.einsum("bnlhd,blhd->bnlh", q, k) * scale
        dist2 = np.sum(inv * inv, axis=-1)
        gw = -dist2 / (2.0 * i["window_sigma"][:, None, :, 0] ** 2)
        att = att + gw[..., None]
        att = att - att.max(axis=2, keepdims=True)
        att = np.exp(att)
        att = att / att.sum(axis=2, keepdims=True)
        y = np.einsum("bnlh,bnlhd->bnhd", att, v).reshape(B, NS, H * D)
        out[:, sl, :] = y @ i["wo"] + i["bo"]
    return out


def kernel(**inputs):
    try:
        return _run_jax(inputs)
    except Exception as e:  # no devices / compile failure -> host fallback
        import sys
        print(f"kernel: device path failed ({type(e).__name__}: {e}); "
              f"using host fallback", file=sys.stderr)
        return _run_numpy(inputs)





# revision 2
# speedup vs baseline: 1.9784x; 1.9784x over previous
"""EquivariantCrossAttention kernel for 8 Trainium2 NeuronCores.

Sharding strategy (per spec hint): the num_coords (N) axis of x / x_h /
output is split 8 ways across the NeuronCores; latents p/a, window_sigma
and all weights are replicated on every core, so the attention reduction
over L stays core-local and needs no collectives.

Host-path optimizations vs. the naive pmap version:
  - the jitted shard_map callable is built once and cached
  - replicated weights are transferred to the devices once (content-hash
    keyed) and reused as device-resident arrays on subsequent calls
  - the device->host result copy is enqueued asynchronously right after
    dispatch so it pipelines with execution instead of costing a separate
    axon round-trip
If the device path is unavailable, a bit-accurate numpy fallback runs on
host.
"""

import hashlib
import os

os.environ.setdefault("NEURON_CC_FLAGS", "--auto-cast=none")

import numpy as np

B, N, L = 2, 2048, 128
H, D = 4, 32
A = 128
C = 2
TWO_PI = 6.283185307179586
NC = 8
NS = N // NC  # 256 coords per core

_REP_KEYS = (
    "p", "a", "window_sigma",
    "wr_q", "w1_q", "b1_q", "w2_q", "b2_q",
    "wr_v", "w1_v", "b1_v", "w2_v", "b2_v",
    "wq", "bq", "wk", "bk", "wv", "bv",
    "cf_w1", "cf_b1", "cf_g", "cf_bt", "cf_w2", "cf_b2",
    "vf_w1", "vf_b1", "vf_g", "vf_bt", "vf_w2", "vf_b2",
    "mf_w1", "mf_b1", "mf_g", "mf_bt", "mf_w2", "mf_b2",
    "wo", "bo",
)


# ---------------------------------------------------------------- jax path
_STATE = {}  # jit fn, mesh, cached device weights


def _build(devs):
    import jax
    import jax.numpy as jnp
    from jax.sharding import Mesh, PartitionSpec as P, NamedSharding
    from jax import shard_map

    def _ln(h, g, b):
        mu = h.mean(-1, keepdims=True)
        var = ((h - mu) ** 2).mean(-1, keepdims=True)
        return (h - mu) * jax.lax.rsqrt(var + 1e-6) * g + b

    def _ffn(x, w1, b1, g, bt, w2, b2):
        h = jax.nn.gelu(x @ w1 + b1)
        return _ln(h, g, bt) @ w2 + b2

    def _emb(inv, wr, w1, b1, w2, b2):
        proj = TWO_PI * (inv @ wr)
        feat = jnp.concatenate([jnp.sin(proj), jnp.cos(proj)], axis=-1)
        return jax.nn.gelu(feat @ w1 + b1) @ w2 + b2

    def shard_fn(x, x_h, r):
        # x: (B, NS, C)  x_h: (B, NS, D); everything in r replicated.
        ns = x.shape[1]
        inv = x[:, :, None, :] - r["p"][:, None, :, :]           # (B,NS,L,C)
        emb_q = _emb(inv, r["wr_q"], r["w1_q"], r["b1_q"],
                     r["w2_q"], r["b2_q"])                        # (B,NS,L,D)
        k = (r["a"] @ r["wk"] + r["bk"]).reshape(B, L, H, D)
        # fold k into wq: att = emb_q @ (wq . k) -- avoids materializing
        # the (B,NS,L,H*D) query tensor (exact reassociation).
        wq3 = r["wq"].reshape(D, H, D)
        wk_f = jnp.einsum("ehd,blhd->belh", wq3, k)               # (B,D,L,H)
        bk_f = jnp.einsum("hd,blhd->blh", r["bq"].reshape(H, D), k)
        v = r["a"] @ r["wv"] + r["bv"]                            # (B,L,H*D)
        inv_emb_v = _emb(inv, r["wr_v"], r["w1_v"], r["b1_v"],
                         r["w2_v"], r["b2_v"])                    # (B,NS,L,D)
        gb = _ffn(x_h, r["cf_w1"], r["cf_b1"], r["cf_g"], r["cf_bt"],
                  r["cf_w2"], r["cf_b2"])                         # (B,NS,2D)
        g_, b_ = jnp.split(gb, 2, axis=-1)
        inv_emb_v = inv_emb_v * (1.0 + g_[:, :, None, :]) + b_[:, :, None, :]
        # vf FFN inlined so the vb half of vf_w2 folds through mf_w1 --
        # vb and the 256-wide vgb are never materialized (exact algebra).
        hv = jax.nn.gelu(inv_emb_v @ r["vf_w1"] + r["vf_b1"])
        hv = _ln(hv, r["vf_g"], r["vf_bt"])                       # (B,NS,L,D)
        vg = hv @ r["vf_w2"][:, :H * D] + r["vf_b2"][:H * D]      # (B,NS,L,HD)
        vfilm = (v[:, None, :, :] * (1.0 + vg)).reshape(B, ns, L, H, D)
        w2b = r["vf_w2"][:, H * D:].reshape(D, H, D)
        w2b_f = jnp.einsum("chd,df->chf", w2b, r["mf_w1"])
        const_f = (jnp.einsum("hd,df->hf",
                              r["vf_b2"][H * D:].reshape(H, D), r["mf_w1"])
                   + r["mf_b1"])                                  # (H,D)
        pre = (jnp.einsum("bnlhd,df->bnlhf", vfilm, r["mf_w1"])
               + jnp.einsum("bnlc,chf->bnlhf", hv, w2b_f) + const_f)
        v = _ln(jax.nn.gelu(pre), r["mf_g"], r["mf_bt"]) @ r["mf_w2"] + r["mf_b2"]
        scale = 1.0 / (D ** 0.5)
        att = (jnp.einsum("bnle,belh->bnlh", emb_q, wk_f)
               + bk_f[:, None]) * scale
        dist2 = jnp.sum(inv * inv, axis=-1)
        gw = -dist2 / (2.0 * r["window_sigma"][:, None, :, 0] ** 2)
        att = att + gw[..., None]
        att = jax.nn.softmax(att, axis=2)
        y = jnp.einsum("bnlh,bnlhd->bnhd", att, v).reshape(B, ns, H * D)
        return y @ r["wo"] + r["bo"]                              # (B,NS,D)

    mesh = Mesh(np.asarray(devs), ("c",))
    # x / x_h arrive stacked (NC*B, NS, ...) so each core's shard is its
    # (B, NS, ...) block; weights are fully replicated.
    def stacked_fn(xs, xhs, r):
        return shard_fn(xs.reshape(B, NS, C), xhs.reshape(B, NS, D), r
                        ).reshape(B * NS, D)

    f = jax.jit(
        shard_map(
            stacked_fn,
            mesh=mesh,
            in_specs=(P("c"), P("c"), P()),
            out_specs=P("c"),
            check_vma=False,
        )
    )
    rep_shard = NamedSharding(mesh, P())
    return f, mesh, rep_shard


def _rep_hash(rep):
    h = hashlib.blake2b(digest_size=16)
    for k in _REP_KEYS:
        h.update(np.ascontiguousarray(rep[k]).tobytes())
    return h.hexdigest()


def _run_jax(inputs):
    import jax

    devs = [d for d in jax.devices() if d.platform != "cpu"][:NC]
    if len(devs) < NC:
        raise RuntimeError(f"need {NC} accelerator devices, got {len(devs)}")

    if "fn" not in _STATE:
        _STATE["fn"], _STATE["mesh"], _STATE["rep_shard"] = _build(devs)
    f = _STATE["fn"]

    rep = {k: np.asarray(inputs[k], dtype=np.float32) for k in _REP_KEYS}
    hsh = _rep_hash(rep)
    if _STATE.get("rep_hash") != hsh:
        rep_dev = jax.device_put(rep, _STATE["rep_shard"])
        jax.block_until_ready(rep_dev)
        _STATE["rep_dev"] = rep_dev
        _STATE["rep_hash"] = hsh

    # stack per-core shards along axis 0: (NC*B, NS, ...)
    x = np.ascontiguousarray(
        inputs["x"].reshape(B, NC, NS, C).transpose(1, 0, 2, 3)
    ).reshape(NC * B, NS, C)
    x_h = np.ascontiguousarray(
        inputs["x_h"].reshape(B, NC, NS, D).transpose(1, 0, 2, 3)
    ).reshape(NC * B, NS, D)

    y = f(x, x_h, _STATE["rep_dev"])          # (NC*B*NS, D) sharded
    try:
        y.copy_to_host_async()
    except Exception:
        pass
    y = np.asarray(y)                          # (NC*B*NS, D)
    y = y.reshape(NC, B, NS, D).transpose(1, 0, 2, 3).reshape(B, N, D)
    return np.ascontiguousarray(y).astype(np.float32)


# -------------------------------------------------------------- numpy path
def _gelu(x):
    # matches jax.nn.gelu(approximate=True)
    x3 = x * x * x
    return (0.5 * x * (1.0 + np.tanh(0.7978845608028654
                                     * (x + 0.044715 * x3)))).astype(np.float32)


def _ln_np(h, g, b):
    mu = h.mean(-1, keepdims=True, dtype=np.float32)
    var = ((h - mu) ** 2).mean(-1, keepdims=True, dtype=np.float32)
    return (h - mu) / np.sqrt(var + 1e-6) * g + b


def _ffn_np(x, w1, b1, g, bt, w2, b2):
    h = _gelu(x @ w1 + b1)
    return _ln_np(h, g, bt) @ w2 + b2


def _emb_np(inv, wr, w1, b1, w2, b2):
    proj = TWO_PI * (inv @ wr)
    feat = np.concatenate([np.sin(proj), np.cos(proj)], axis=-1)
    return _gelu(feat @ w1 + b1) @ w2 + b2


def _run_numpy(inputs):
    i = {k: np.asarray(v, dtype=np.float32) for k, v in inputs.items()}
    out = np.empty((B, N, D), dtype=np.float32)
    k = (i["a"] @ i["wk"] + i["bk"]).reshape(B, L, H, D)
    v0 = i["a"] @ i["wv"] + i["bv"]
    gb_full = _ffn_np(i["x_h"], i["cf_w1"], i["cf_b1"], i["cf_g"],
                      i["cf_bt"], i["cf_w2"], i["cf_b2"])
    scale = 1.0 / (D ** 0.5)
    for s in range(NC):  # per-shard to bound memory
        sl = slice(s * NS, (s + 1) * NS)
        inv = i["x"][:, sl, None, :] - i["p"][:, None, :, :]
        q = _emb_np(inv, i["wr_q"], i["w1_q"], i["b1_q"], i["w2_q"], i["b2_q"])
        q = (q @ i["wq"] + i["bq"]).reshape(B, NS, L, H, D)
        iev = _emb_np(inv, i["wr_v"], i["w1_v"], i["b1_v"], i["w2_v"], i["b2_v"])
        g_ = gb_full[:, sl, :D]
        b_ = gb_full[:, sl, D:]
        iev = iev * (1.0 + g_[:, :, None, :]) + b_[:, :, None, :]
        vgb = _ffn_np(iev, i["vf_w1"], i["vf_b1"], i["vf_g"], i["vf_bt"],
                      i["vf_w2"], i["vf_b2"])
        vg, vb = vgb[..., :H * D], vgb[..., H * D:]
        v = v0[:, None, :, :] * (1.0 + vg) + vb
        v = _ffn_np(v.reshape(B, NS, L, H, D), i["mf_w1"], i["mf_b1"],
                    i["mf_g"], i["mf_bt"], i["mf_w2"], i["mf_b2"])
        att = np.einsum("bnlhd,blhd->bnlh", q, k) * scale
        dist2 = np.sum(inv * inv, axis=-1)
        gw = -dist2 / (2.0 * i["window_sigma"][:, None, :, 0] ** 2)
        att = att + gw[..., None]
        att = att - att.max(axis=2, keepdims=True)
        att = np.exp(att)
        att = att / att.sum(axis=2, keepdims=True)
        y = np.einsum("bnlh,bnlhd->bnhd", att, v).reshape(B, NS, H * D)
        out[:, sl, :] = y @ i["wo"] + i["bo"]
    return out


def kernel(**inputs):
    try:
        return _run_jax(inputs)
    except Exception as e:  # no devices / compile failure -> host fallback
        import sys
        print(f"kernel: device path failed ({type(e).__name__}: {e}); "
              f"using host fallback", file=sys.stderr)
        return _run_numpy(inputs)


# revision 4
# speedup vs baseline: 2.1036x; 1.0633x over previous
"""EquivariantCrossAttention kernel for 8 Trainium2 NeuronCores.

Sharding strategy (per spec hint): the num_coords (N) axis of x / x_h /
output is split 8 ways across the NeuronCores; latents p/a, window_sigma
and all weights are replicated on every core, so the attention reduction
over L stays core-local and needs no collectives.

Host-path optimizations vs. the naive pmap version:
  - the jitted shard_map callable is built once and cached
  - replicated weights are transferred to the devices once (content-hash
    keyed) and reused as device-resident arrays on subsequent calls
  - the device->host result copy is enqueued asynchronously right after
    dispatch so it pipelines with execution instead of costing a separate
    axon round-trip
If the device path is unavailable, a bit-accurate numpy fallback runs on
host.
"""

import hashlib
import os

os.environ.setdefault("NEURON_CC_FLAGS", "--auto-cast=none")

import numpy as np

B, N, L = 2, 2048, 128
H, D = 4, 32
A = 128
C = 2
TWO_PI = 6.283185307179586
NC = 8
NS = N // NC  # 256 coords per core

_REP_KEYS = (
    "p", "a", "window_sigma",
    "wr_q", "w1_q", "b1_q", "w2_q", "b2_q",
    "wr_v", "w1_v", "b1_v", "w2_v", "b2_v",
    "wq", "bq", "wk", "bk", "wv", "bv",
    "cf_w1", "cf_b1", "cf_g", "cf_bt", "cf_w2", "cf_b2",
    "vf_w1", "vf_b1", "vf_g", "vf_bt", "vf_w2", "vf_b2",
    "mf_w1", "mf_b1", "mf_g", "mf_bt", "mf_w2", "mf_b2",
    "wo", "bo",
)


# ---------------------------------------------------------------- jax path
_STATE = {}  # jit fn, mesh, cached device weights


def _build(devs):
    import jax
    import jax.numpy as jnp
    from jax.sharding import Mesh, PartitionSpec as P, NamedSharding
    from jax import shard_map

    def _ln(h, g, b):
        mu = h.mean(-1, keepdims=True)
        var = ((h - mu) ** 2).mean(-1, keepdims=True)
        return (h - mu) * jax.lax.rsqrt(var + 1e-6) * g + b

    def _ffn(x, w1, b1, g, bt, w2, b2):
        h = jax.nn.gelu(x @ w1 + b1)
        return _ln(h, g, bt) @ w2 + b2

    def _emb(inv, wr, w1, b1, w2, b2):
        proj = TWO_PI * (inv @ wr)
        feat = jnp.concatenate([jnp.sin(proj), jnp.cos(proj)], axis=-1)
        return jax.nn.gelu(feat @ w1 + b1) @ w2 + b2

    def shard_fn(x, x_h, r):
        # x: (B, NS, C)  x_h: (B, NS, D); everything in r replicated.
        ns = x.shape[1]
        inv = x[:, :, None, :] - r["p"][:, None, :, :]           # (B,NS,L,C)
        emb_q = _emb(inv, r["wr_q"], r["w1_q"], r["b1_q"],
                     r["w2_q"], r["b2_q"])                        # (B,NS,L,D)
        k = (r["a"] @ r["wk"] + r["bk"]).reshape(B, L, H, D)
        # fold k into wq: att = emb_q @ (wq . k) -- avoids materializing
        # the (B,NS,L,H*D) query tensor (exact reassociation).
        wq3 = r["wq"].reshape(D, H, D)
        wk_f = jnp.einsum("ehd,blhd->belh", wq3, k)               # (B,D,L,H)
        bk_f = jnp.einsum("hd,blhd->blh", r["bq"].reshape(H, D), k)
        v = r["a"] @ r["wv"] + r["bv"]                            # (B,L,H*D)
        inv_emb_v = _emb(inv, r["wr_v"], r["w1_v"], r["b1_v"],
                         r["w2_v"], r["b2_v"])                    # (B,NS,L,D)
        gb = _ffn(x_h, r["cf_w1"], r["cf_b1"], r["cf_g"], r["cf_bt"],
                  r["cf_w2"], r["cf_b2"])                         # (B,NS,2D)
        g_, b_ = jnp.split(gb, 2, axis=-1)
        inv_emb_v = inv_emb_v * (1.0 + g_[:, :, None, :]) + b_[:, :, None, :]
        # vf FFN inlined so the vb half of vf_w2 folds through mf_w1 --
        # vb and the 256-wide vgb are never materialized (exact algebra).
        hv = jax.nn.gelu(inv_emb_v @ r["vf_w1"] + r["vf_b1"])
        hv = _ln(hv, r["vf_g"], r["vf_bt"])                       # (B,NS,L,D)
        vg = hv @ r["vf_w2"][:, :H * D] + r["vf_b2"][:H * D]      # (B,NS,L,HD)
        vfilm = (v[:, None, :, :] * (1.0 + vg)).reshape(B, ns, L, H, D)
        w2b = r["vf_w2"][:, H * D:].reshape(D, H, D)
        w2b_f = jnp.einsum("chd,df->chf", w2b, r["mf_w1"])
        const_f = (jnp.einsum("hd,df->hf",
                              r["vf_b2"][H * D:].reshape(H, D), r["mf_w1"])
                   + r["mf_b1"])                                  # (H,D)
        pre = (jnp.einsum("bnlhd,df->bnlhf", vfilm, r["mf_w1"])
               + jnp.einsum("bnlc,chf->bnlhf", hv, w2b_f) + const_f)
        v = _ln(jax.nn.gelu(pre), r["mf_g"], r["mf_bt"]) @ r["mf_w2"] + r["mf_b2"]
        scale = 1.0 / (D ** 0.5)
        att = (jnp.einsum("bnle,belh->bnlh", emb_q, wk_f)
               + bk_f[:, None]) * scale
        dist2 = jnp.sum(inv * inv, axis=-1)
        gw = -dist2 / (2.0 * r["window_sigma"][:, None, :, 0] ** 2)
        att = att + gw[..., None]
        att = jax.nn.softmax(att, axis=2)
        y = jnp.einsum("bnlh,bnlhd->bnhd", att, v).reshape(B, ns, H * D)
        return y @ r["wo"] + r["bo"]                              # (B,NS,D)

    mesh = Mesh(np.asarray(devs), ("c",))
    # x and x_h ride in one stacked (NC*B, NS, C+D) tensor so each call
    # costs a single host->device transfer; each core's shard is its
    # (B, NS, C+D) block. Weights are fully replicated.
    def stacked_fn(xc, r):
        xc = xc.reshape(B, NS, C + D)
        return shard_fn(xc[:, :, :C], xc[:, :, C:], r).reshape(B * NS, D)

    f = jax.jit(
        shard_map(
            stacked_fn,
            mesh=mesh,
            in_specs=(P("c"), P()),
            out_specs=P("c"),
            check_vma=False,
        )
    )
    rep_shard = NamedSharding(mesh, P())
    return f, mesh, rep_shard


def _rep_hash(rep):
    h = hashlib.blake2b(digest_size=16)
    for k in _REP_KEYS:
        h.update(np.ascontiguousarray(rep[k]).tobytes())
    return h.hexdigest()


def _run_jax(inputs):
    import jax

    devs = [d for d in jax.devices() if d.platform != "cpu"][:NC]
    if len(devs) < NC:
        raise RuntimeError(f"need {NC} accelerator devices, got {len(devs)}")

    if "fn" not in _STATE:
        _STATE["fn"], _STATE["mesh"], _STATE["rep_shard"] = _build(devs)
    f = _STATE["fn"]

    rep = {k: np.asarray(inputs[k], dtype=np.float32) for k in _REP_KEYS}
    hsh = _rep_hash(rep)
    if _STATE.get("rep_hash") != hsh:
        rep_dev = jax.device_put(rep, _STATE["rep_shard"])
        jax.block_until_ready(rep_dev)
        _STATE["rep_dev"] = rep_dev
        _STATE["rep_hash"] = hsh

    # stack per-core shards along axis 0 into one upload: (NC*B, NS, C+D)
    xc = np.empty((NC, B, NS, C + D), dtype=np.float32)
    xc[:, :, :, :C] = np.asarray(inputs["x"], np.float32).reshape(
        B, NC, NS, C).transpose(1, 0, 2, 3)
    xc[:, :, :, C:] = np.asarray(inputs["x_h"], np.float32).reshape(
        B, NC, NS, D).transpose(1, 0, 2, 3)
    xc = xc.reshape(NC * B, NS, C + D)

    y = f(xc, _STATE["rep_dev"])              # (NC*B*NS, D) sharded
    try:
        y.copy_to_host_async()
    except Exception:
        pass
    y = np.asarray(y)                          # (NC*B*NS, D)
    y = y.reshape(NC, B, NS, D).transpose(1, 0, 2, 3).reshape(B, N, D)
    return np.ascontiguousarray(y).astype(np.float32)


# -------------------------------------------------------------- numpy path
def _gelu(x):
    # matches jax.nn.gelu(approximate=True)
    x3 = x * x * x
    return (0.5 * x * (1.0 + np.tanh(0.7978845608028654
                                     * (x + 0.044715 * x3)))).astype(np.float32)


def _ln_np(h, g, b):
    mu = h.mean(-1, keepdims=True, dtype=np.float32)
    var = ((h - mu) ** 2).mean(-1, keepdims=True, dtype=np.float32)
    return (h - mu) / np.sqrt(var + 1e-6) * g + b


def _ffn_np(x, w1, b1, g, bt, w2, b2):
    h = _gelu(x @ w1 + b1)
    return _ln_np(h, g, bt) @ w2 + b2


def _emb_np(inv, wr, w1, b1, w2, b2):
    proj = TWO_PI * (inv @ wr)
    feat = np.concatenate([np.sin(proj), np.cos(proj)], axis=-1)
    return _gelu(feat @ w1 + b1) @ w2 + b2


def _run_numpy(inputs):
    i = {k: np.asarray(v, dtype=np.float32) for k, v in inputs.items()}
    out = np.empty((B, N, D), dtype=np.float32)
    k = (i["a"] @ i["wk"] + i["bk"]).reshape(B, L, H, D)
    v0 = i["a"] @ i["wv"] + i["bv"]
    gb_full = _ffn_np(i["x_h"], i["cf_w1"], i["cf_b1"], i["cf_g"],
                      i["cf_bt"], i["cf_w2"], i["cf_b2"])
    scale = 1.0 / (D ** 0.5)
    for s in range(NC):  # per-shard to bound memory
        sl = slice(s * NS, (s + 1) * NS)
        inv = i["x"][:, sl, None, :] - i["p"][:, None, :, :]
        q = _emb_np(inv, i["wr_q"], i["w1_q"], i["b1_q"], i["w2_q"], i["b2_q"])
        q = (q @ i["wq"] + i["bq"]).reshape(B, NS, L, H, D)
        iev = _emb_np(inv, i["wr_v"], i["w1_v"], i["b1_v"], i["w2_v"], i["b2_v"])
        g_ = gb_full[:, sl, :D]
        b_ = gb_full[:, sl, D:]
        iev = iev * (1.0 + g_[:, :, None, :]) + b_[:, :, None, :]
        vgb = _ffn_np(iev, i["vf_w1"], i["vf_b1"], i["vf_g"], i["vf_bt"],
                      i["vf_w2"], i["vf_b2"])
        vg, vb = vgb[..., :H * D], vgb[..., H * D:]
        v = v0[:, None, :, :] * (1.0 + vg) + vb
        v = _ffn_np(v.reshape(B, NS, L, H, D), i["mf_w1"], i["mf_b1"],
                    i["mf_g"], i["mf_bt"], i["mf_w2"], i["mf_b2"])
        att = np.einsum("bnlhd,blhd->bnlh", q, k) * scale
        dist2 = np.sum(inv * inv, axis=-1)
        gw = -dist2 / (2.0 * i["window_sigma"][:, None, :, 0] ** 2)
        att = att + gw[..., None]
        att = att - att.max(axis=2, keepdims=True)
        att = np.exp(att)
        att = att / att.sum(axis=2, keepdims=True)
        y = np.einsum("bnlh,bnlhd->bnhd", att, v).reshape(B, NS, H * D)
        out[:, sl, :] = y @ i["wo"] + i["bo"]
    return out


def kernel(**inputs):
    try:
        return _run_jax(inputs)
    except Exception as e:  # no devices / compile failure -> host fallback
        import sys
        print(f"kernel: device path failed ({type(e).__name__}: {e}); "
              f"using host fallback", file=sys.stderr)
        return _run_numpy(inputs)
